# revision 45
# baseline (speedup 1.0000x reference)
"""Trainium2 Bass kernel for nn_DecoderCell (LFADS-style decoder cell).

Strategy: pure data parallel over 8 NeuronCores (batch 32768 -> 4096/core),
feature-major [feat, batch] on device. Per-core loop over 4 PAIRS of batch
tiles (N=512 each, pair width 1024 for DMA/elementwise), phased (all-con,
all-co, all-gen, all-fac) so the Act engine reloads the sigmoid/tanh and
exp activation tables only twice per pass.

Precision plan (validated op-by-op against the reference in numpy,
rel 1.79e-2 on HW vs the 2e-2 gate — numpy model matches HW exactly):
ALL GRU matmuls run as fp8e4m3 DoubleRow pairs (0.5 cyc/row on the PE).
The x-paths recover near-bf16 accuracy via fp8 RESIDUAL terms:
  con x:  W8@(x8 + r8)        (data residual r8 = fp8(x - fp8(x)))
  gen x:  (W8 + Wr8)@(g8 + gr8)  (data + weight residuals; the W8/Wr8
          passes re-read the same (g8, gr8) rhs chunk pair)
The residual chunks ride in the same contraction as the main data, so a
con-zr output chunk costs 4 DR passes (x8 x3 chunks + r8 x3 + h8 x2 = 8
chunks) instead of 3 bf16 + 1 DR. co_linear and factor stay bf16. PSUM
accumulation is fp32; activations are f32-in/bf16-out, consolidated into
4-bank PSUM groups (one act instr per 2048-elem group).

Engine/queue balance (v1 cost model charges DMA transfers to the issuing
queue): all large stream DMAs issue from the SP queue; od_msg/od_gi/od_fac
from the Pool queue; weight loads round-robin SP/Pool/Act.  Elementwise is
split DVE/Pool per the KNOBS: r*h products write fp8 straight into the
combined rhs tiles (cxh chunks 8-9 / gh chunks 6-9); the n-stage x-partial
matmuls are issued BEFORE the r*h products so the PE never waits on them.

DMA I/O: bf16 h-state + fp8 combined streams, tile-pair-major in dram so
every stream DMA is one contiguous-per-partition transfer. Outputs are
written bf16 and upcast on the host; the +-5 clip is applied host-side
(only |h_0|>5 elements can clip; in-tolerance downstream).

Cost-model steady state: ~105us/rep/core (PE 72-75us, Act 76us, DVE 62us,
SP 63us, Pool 40us) vs ~151us for the session-start baseline.
"""

import sys

sys.path.insert(0, "/opt/trn_rl_repo")

import numpy as np

import concourse.bacc as bacc
import concourse.tile as tile
import concourse.mybir as mybir

N_CORES = 8
BATCH = 32768
B_CORE = BATCH // N_CORES  # 4096
NT = 512                   # batch tile (free dim per matmul / PSUM bank)
NTILES = B_CORE // NT      # 8
NP = NTILES // 2           # 4 tile-pairs
PW = 2 * NT                # 1024 pair width

GEN, CON, CO, FAC, CIE, EXT = 512, 256, 64, 128, 128, 16
X_DIM = 2 * CIE + EXT      # 272
H_DIM = GEN + CON + 3 * CO + EXT + FAC  # 1104
CLIP = 5.0

F32 = mybir.dt.float32
BF = mybir.dt.bfloat16
F8 = mybir.dt.float8e4
E3 = mybir.dt.float8e3
AF = mybir.ActivationFunctionType
ALU = mybir.AluOpType
DR = mybir.MatmulPerfMode.DoubleRow

KNOBS = {
    "io_bufs": 2,
    "act_bufs": 2,
    "p4_bufs": 2,
    "hc_bufs": 1,
    "gix_bufs": 1,
    "gate_exp": False,     # zero-weight matmul gating co psum on last con tanh
    "gate_a": False,       # gate next rep's first con zr groups on last gix
    "gate_pair": 1,       # which pair's tanh gates the exps
    # engine for each elementwise op: 'v' = DVE, 'g' = Pool/gpsimd
    "mean_cp": "v",
    "fc_cp": "v",
    "con_rh0": "v",
    "con_rh1": "g",
    "con_d": "g",
    "con_zd": "v",
    "con_nd": "v",
    "gen_rh0": "v",
    "gen_rh1": "g",
    "gen_d0": "v",
    "gen_d1": "g",
    "gen_zd0": "v",
    "gen_zd1": "v",
    "gen_nd0": "v",
    "gen_nd1": "v",
    # which engine queue issues each DMA: sync=SP, gpsimd=Pool
    "in_q": "sync",
    "out_q": "sync",
    "out_q2": "gpsimd",
    "co_ilv": False,
    "fac_delay": False,
}


def ts(i, s):
    return slice(i * s, (i + 1) * s)


def build_program(has_bias: bool, repeat: int = 1, num_devices: int = N_CORES):
    nc = bacc.Bacc("TRN2", target_bir_lowering=False, debug=False,
                   num_devices=num_devices)

    # ---- per-core streaming inputs, tile-pair-major [NP, P, flat] ----
    # cxh8d chunks: 0..2 x8 (ci+factor fp8), 3..5 xr8 (fp8 residual),
    # 6..7 h8_con.  gx8d (partitions 64..127 of gh chunks 4..5):
    # ext8/extr8 on rows 0..15, zeros elsewhere.
    cxh8d = nc.dram_tensor("cxh8d", [NP, 128, 8 * PW], F8,
                           kind="ExternalInput")
    hc16d = nc.dram_tensor("hc16d", [NP, 128, 2 * PW], BF,
                           kind="ExternalInput")
    hg16d = nc.dram_tensor("hg16d", [NP, 128, 4 * PW], BF,
                           kind="ExternalInput")
    hg8d = nc.dram_tensor("hg8d", [NP, 128, 4 * PW], F8,
                          kind="ExternalInput")
    gx8d = nc.dram_tensor("gx8d", [NP, 64, 2 * PW], F8,
                          kind="ExternalInput")
    epsd = nc.dram_tensor("epsd", [NP, 64, PW], BF, kind="ExternalInput")
    # weights, pre-packed on host into the SBUF layouts (see _prep_inputs)
    wczr8d = nc.dram_tensor("wczr8d", [128, 8, 512], F8,
                            kind="ExternalInput")
    wcn8d = nc.dram_tensor("wcn8d", [128, 8, 256], F8, kind="ExternalInput")
    wcod = nc.dram_tensor("wcod", [128, 2, 128], BF, kind="ExternalInput")
    wgzr8d = nc.dram_tensor("wgzr8d", [128, 8, 1024], F8,
                            kind="ExternalInput")
    wgn8d = nc.dram_tensor("wgn8d", [128, 8, 512], F8, kind="ExternalInput")
    wfacd = nc.dram_tensor("wfacd", [128, 4, 128], BF, kind="ExternalInput")
    if has_bias:
        bczrd = nc.dram_tensor("bczrd", [128, 4], F32, kind="ExternalInput")
        bcnd = nc.dram_tensor("bcnd", [128, 2], F32, kind="ExternalInput")
        bcod = nc.dram_tensor("bcod", [64, 2], F32, kind="ExternalInput")
        bgzrd = nc.dram_tensor("bgzrd", [128, 8], F32, kind="ExternalInput")
        bgnd = nc.dram_tensor("bgnd", [128, 4], F32, kind="ExternalInput")

    # outputs, tile-pair-major
    od_gen = nc.dram_tensor("od_gen", [NP, 128, 4 * PW], BF,
                            kind="ExternalOutput")
    od_con = nc.dram_tensor("od_con", [NP, 128, 2 * PW], BF,
                            kind="ExternalOutput")
    od_msg = nc.dram_tensor("od_msg", [NP, 64, 2 * PW], BF,
                            kind="ExternalOutput")
    od_gi = nc.dram_tensor("od_gi", [NP, 64, PW], BF, kind="ExternalOutput")
    od_fac = nc.dram_tensor("od_fac", [NP, 128, PW], BF,
                            kind="ExternalOutput")

    with tile.TileContext(nc) as tc:
        with (
            tc.tile_pool(name="w", bufs=1) as wp,
            tc.tile_pool(name="io", bufs=KNOBS["io_bufs"]) as io,
            tc.tile_pool(name="act", bufs=KNOBS["act_bufs"]) as act,
            tc.tile_pool(name="hcp", bufs=1) as hcp,
            tc.tile_pool(name="gixp", bufs=1) as gixp,
            tc.tile_pool(name="p4", bufs=KNOBS["p4_bufs"], space="PSUM") as p4,
        ):
            ENG = {"v": nc.vector, "g": nc.gpsimd}
            INQ = getattr(nc, KNOBS["in_q"])
            OUTQ = getattr(nc, KNOBS["out_q"])
            OUTQ2 = getattr(nc, KNOBS["out_q2"])

            _wq = [nc.sync, nc.gpsimd, nc.scalar, nc.gpsimd]
            _wqi = [0]

            def wload(dram, shape, dt, tag):
                t = wp.tile(shape, dt, tag=tag)
                _wq[_wqi[0] % 4].dma_start(out=t, in_=dram[:, :, :])
                _wqi[0] += 1
                return t

            Wczr = wload(wczr8d, [128, 8, 512], F8, "Wczr")
            Wcn = wload(wcn8d, [128, 8, 256], F8, "Wcn")
            Wco = wload(wcod, [128, 2, 128], BF, "Wco")
            Wgzr = wload(wgzr8d, [128, 8, 1024], F8, "Wgzr")
            Wgn = wload(wgn8d, [128, 8, 512], F8, "Wgn")
            Wfac = wload(wfacd, [128, 4, 128], BF, "Wfac")
            if KNOBS["gate_exp"] or KNOBS["gate_a"]:
                Zw = wp.tile([128, 128], BF, tag="Zw")
                nc.vector.memset(Zw, 0)
            if has_bias:
                Bczr = wp.tile([128, 4], F32, tag="Bczr")
                nc.sync.dma_start(out=Bczr, in_=bczrd[:, :])
                Bcn = wp.tile([128, 2], F32, tag="Bcn")
                nc.sync.dma_start(out=Bcn, in_=bcnd[:, :])
                Bco = wp.tile([64, 2], F32, tag="Bco")
                nc.sync.dma_start(out=Bco, in_=bcod[:, :])
                Bgzr = wp.tile([128, 8], F32, tag="Bgzr")
                nc.sync.dma_start(out=Bgzr, in_=bgzrd[:, :])
                Bgn = wp.tile([128, 4], F32, tag="Bgn")
                nc.sync.dma_start(out=Bgn, in_=bgnd[:, :])

            def act_write(dst_tile, c0, t, psum, nch, func, bias_tile,
                          bias_c0):
                # dst_tile: [128, C, 2, NT]; writes chunks c0:c0+nch, tile t
                if has_bias:
                    for c in range(nch):
                        nc.scalar.activation(
                            dst_tile[:, c0 + c, t, :], psum[:, c, :], func,
                            bias=bias_tile[:, bias_c0 + c:bias_c0 + c + 1])
                else:
                    nc.scalar.activation(dst_tile[:, c0:c0 + nch, t, :],
                                         psum[:, 0:nch, :], func)

            def split2(op_name, dst, a, b, k0, k1, nch):
                """Tensor-tensor op over [128, nch, 2, 512] pair tensors,
                split into chunk halves across engines; one instr if same."""
                h = nch // 2
                if k0 == k1:
                    getattr(ENG[k0], op_name)(
                        dst[:, 0:nch, :, :], a[:, 0:nch, :, :],
                        b[:, 0:nch, :, :])
                else:
                    getattr(ENG[k0], op_name)(
                        dst[:, 0:h, :, :], a[:, 0:h, :, :], b[:, 0:h, :, :])
                    getattr(ENG[k1], op_name)(
                        dst[:, h:nch, :, :], a[:, h:nch, :, :],
                        b[:, h:nch, :, :])

            # ---------------- stages (per tile-pair) ----------------
            def con_fetch(p):
                # cxh chunks: 0..2 x8, 3..5 xr8, 6..7 h8, 8..9 rh8 (DVE)
                cxh = io.tile([128, 10, 2, NT], F8, tag="cxh", bufs=2)
                INQ.dma_start(
                    out=cxh[:, 0:8, :, :],
                    in_=cxh8d[p].rearrange("p (c t n) -> p c t n", c=8, t=2))
                h16 = hcp.tile([128, 2, 2, NT], BF, tag=f"hc{p}",
                               bufs=KNOBS["hc_bufs"])
                INQ.dma_start(
                    out=h16,
                    in_=hc16d[p].rearrange("p (c t n) -> p c t n", c=2, t=2))
                return cxh, h16

            def con_stage(p, fetched, gate_src=None):
                cxh, h16 = fetched
                zr = act.tile([128, 4, 2, NT], BF, tag="zr_c")
                for t in range(2):
                    pz = p4.tile([128, 4, NT], F32, tag="p4")
                    for m in range(4):
                        for pi in range(4):
                            nc.tensor.matmul(pz[:, m, :],
                                             Wczr[:, 2 * pi:2 * pi + 2,
                                                  ts(m, 128)],
                                             cxh[:, 2 * pi:2 * pi + 2, t, :],
                                             start=(pi == 0), stop=(pi == 3),
                                             perf_mode=DR)
                    act_write(zr, 0, t, pz, 4, AF.Sigmoid,
                              has_bias and Bczr, 0)

                # rh8 lands in cxh chunks 8..9
                for t in range(2):
                    e = KNOBS["con_rh0"] if t == 0 else KNOBS["con_rh1"]
                    ENG[e].tensor_mul(cxh[:, 8:10, t, :],
                                      zr[:, 2:4, t, :],
                                      h16[:, :, t, :])

                # x-partials first (independent of rh), then the rh DR passes
                n_t = act.tile([128, 2, 2, NT], BF, tag="n_c")
                pns = []
                for t in range(2):
                    pn = p4.tile([128, 4, NT], F32, tag="p4")
                    pns.append(pn)
                    for mi in range(2):
                        for pi in range(3):
                            nc.tensor.matmul(pn[:, mi, :],
                                             Wcn[:, 2 * pi:2 * pi + 2,
                                                 ts(mi, 128)],
                                             cxh[:, 2 * pi:2 * pi + 2, t, :],
                                             start=(pi == 0), stop=False,
                                             perf_mode=DR)
                for t in range(2):
                    pn = pns[t]
                    for mi in range(2):
                        nc.tensor.matmul(pn[:, mi, :],
                                         Wcn[:, 6:8, ts(mi, 128)],
                                         cxh[:, 8:10, t, :],
                                         start=False, stop=True, perf_mode=DR)
                    act_write(n_t, 0, t, pn, 2, AF.Tanh, has_bias and Bcn, 0)

                # h' = n + z*(h - n), in place on h16 (clip on host)
                ENG[KNOBS["con_d"]].tensor_sub(h16, h16[:, :, :, :],
                                               n_t[:, :, :, :])
                ENG[KNOBS["con_zd"]].tensor_mul(h16, zr[:, 0:2, :, :],
                                                h16[:, :, :, :])
                ENG[KNOBS["con_nd"]].tensor_add(h16, n_t[:, :, :, :],
                                                h16[:, :, :, :])
                OUTQ.dma_start(
                    out=od_con[p].rearrange("p (c t n) -> p c t n", c=2, t=2),
                    in_=h16)
                return h16, n_t

            def co_stage(p, h16, gate_nt, epsall):
                epst = io.tile([64, 2, NT], BF, tag="epst", bufs=2)
                INQ.dma_start(
                    out=epst,
                    in_=epsd[p].rearrange("p (u n) -> p u n", u=2))
                msg = io.tile([64, 2, 2, NT], BF, tag="msg", bufs=2)
                P = p4.tile([128, 4, NT], F32, tag="p4")
                for t in range(2):
                    for k in range(2):
                        nc.tensor.matmul(P[:, t, :], Wco[:, k, :],
                                         h16[:, k, t, :],
                                         start=(k == 0), stop=(k == 1))
                if has_bias:
                    for t in range(2):
                        nc.scalar.activation(msg[:, 0, t, :], P[0:64, t, :],
                                             AF.Identity, bias=Bco[:, 0:1])
                        nc.scalar.activation(msg[:, 1, t, :], P[64:128, t, :],
                                             AF.Exp, scale=0.5,
                                             bias=Bco[:, 1:2])
                else:
                    ENG[KNOBS["mean_cp"]].tensor_copy(msg[:, 0, :, :],
                                                      P[0:64, 0:2, :])
                    nc.scalar.activation(msg[:, 1, :, :], P[64:128, 0:2, :],
                                         AF.Exp, scale=0.5)
                gi = gixp.tile([64, 2, NT], BF, tag=f"gix{p}",
                               bufs=KNOBS["gix_bufs"])
                for t in range(2):
                    ep = epst[:, t, :]
                    nc.vector.tensor_mul(ep, msg[:, 1, t, :], ep)
                    nc.vector.tensor_add(gi[:, t, :], msg[:, 0, t, :], ep)
                OUTQ2.dma_start(
                    out=od_msg[p].rearrange("p (c t n) -> p c t n", c=2, t=2),
                    in_=msg)
                OUTQ2.dma_start(
                    out=od_gi[p].rearrange("p (t n) -> p t n", t=2),
                    in_=gi)
                return gi

            def gen_fetch(p):
                hg16 = io.tile([128, 4, 2, NT], BF, tag="hg16", bufs=4)
                INQ.dma_start(
                    out=hg16,
                    in_=hg16d[p].rearrange("p (c t n) -> p c t n", c=4, t=2))
                # gh chunks: 0..3 hg8, 4 g8, 5 gr8, 6..9 rg8
                gh = io.tile([128, 10, 2, NT], F8, tag="gh8", bufs=2)
                INQ.dma_start(
                    out=gh[:, 0:4, :, :],
                    in_=hg8d[p].rearrange("p (c t n) -> p c t n", c=4, t=2))
                # ext8/extr8 rows (+ zero padding) for partitions 64..127
                INQ.dma_start(
                    out=gh[64:128, 4:6, :, :],
                    in_=gx8d[p].rearrange("p (c t n) -> p c t n", c=2, t=2))
                return hg16, gh

            def gen_stage(p, gi, fetched):
                hg16, gh = fetched
                # g8 / gr8 into gh chunks 4, 5 (partitions 0..63)
                nc.vector.tensor_copy(gh[0:64, 4, :, :], gi)
                nc.gpsimd.tensor_sub(gh[0:64, 5, :, :], gi,
                                     gh[0:64, 4, :, :])
                zg = act.tile([128, 4, 2, NT], BF, tag="zg")
                rg16 = act.tile([128, 4, 2, NT], BF, tag="rg16")
                for t in range(2):
                    for g in range(2):
                        pz = p4.tile([128, 4, NT], F32, tag="p4")
                        # h passes first (independent of the co-derived g8)
                        for mi in range(4):
                            m = 4 * g + mi
                            nc.tensor.matmul(pz[:, mi, :],
                                             Wgzr[:, 0:2, ts(m, 128)],
                                             gh[:, 0:2, t, :],
                                             start=True, stop=False,
                                             perf_mode=DR)
                            nc.tensor.matmul(pz[:, mi, :],
                                             Wgzr[:, 2:4, ts(m, 128)],
                                             gh[:, 2:4, t, :],
                                             start=False, stop=False,
                                             perf_mode=DR)
                        for mi in range(4):
                            m = 4 * g + mi
                            nc.tensor.matmul(pz[:, mi, :],
                                             Wgzr[:, 4:6, ts(m, 128)],
                                             gh[:, 4:6, t, :],
                                             start=False, stop=False,
                                             perf_mode=DR)
                            nc.tensor.matmul(pz[:, mi, :],
                                             Wgzr[:, 6:8, ts(m, 128)],
                                             gh[:, 4:6, t, :],
                                             start=False, stop=True,
                                             perf_mode=DR)
                        dst = zg if g == 0 else rg16
                        act_write(dst, 0, t, pz, 4, AF.Sigmoid,
                                  has_bias and Bgzr, 4 * g)

                # rg8 lands in gh chunks 6..9; per-t halves on both engines
                for t in range(2):
                    e = KNOBS["gen_rh0"] if t == 0 else KNOBS["gen_rh1"]
                    ENG[e].tensor_mul(gh[:, 6:10, t, :], rg16[:, :, t, :],
                                      hg16[:, :, t, :])

                # x-partials first (independent of rg8), then the DR passes
                ng = act.tile([128, 4, 2, NT], BF, tag="ng", bufs=2)
                pns = []
                for t in range(2):
                    pn = p4.tile([128, 4, NT], F32, tag="p4")
                    pns.append(pn)
                    for m in range(4):
                        nc.tensor.matmul(pn[:, m, :],
                                         Wgn[:, 0:2, ts(m, 128)],
                                         gh[:, 4:6, t, :],
                                         start=True, stop=False,
                                         perf_mode=DR)
                        nc.tensor.matmul(pn[:, m, :],
                                         Wgn[:, 2:4, ts(m, 128)],
                                         gh[:, 4:6, t, :],
                                         start=False, stop=False,
                                         perf_mode=DR)
                for t in range(2):
                    pn = pns[t]
                    for m in range(4):
                        nc.tensor.matmul(pn[:, m, :],
                                         Wgn[:, 4:6, ts(m, 128)],
                                         gh[:, 6:8, t, :],
                                         start=False, stop=False,
                                         perf_mode=DR)
                        nc.tensor.matmul(pn[:, m, :],
                                         Wgn[:, 6:8, ts(m, 128)],
                                         gh[:, 8:10, t, :],
                                         start=False, stop=True,
                                         perf_mode=DR)
                    act_write(ng, 0, t, pn, 4, AF.Tanh,
                              has_bias and Bgn, 0)

                split2("tensor_sub", hg16, hg16, ng,
                       KNOBS["gen_d0"], KNOBS["gen_d1"], 4)
                split2("tensor_mul", hg16, zg, hg16,
                       KNOBS["gen_zd0"], KNOBS["gen_zd1"], 4)
                split2("tensor_add", hg16, ng, hg16,
                       KNOBS["gen_nd0"], KNOBS["gen_nd1"], 4)
                OUTQ.dma_start(
                    out=od_gen[p].rearrange("p (c t n) -> p c t n", c=4, t=2),
                    in_=hg16)
                return hg16

            def fac_stage(p, hg16):
                fc = io.tile([128, 2, NT], BF, tag="fc", bufs=2)
                pf = p4.tile([128, 4, NT], F32, tag="p4")
                for t in range(2):
                    for k in range(4):
                        nc.tensor.matmul(pf[:, t, :], Wfac[:, k, :],
                                         hg16[:, k, t, :],
                                         start=(k == 0), stop=(k == 3))
                ENG[KNOBS["fc_cp"]].tensor_copy(fc, pf[:, 0:2, :])
                OUTQ2.dma_start(
                    out=od_fac[p].rearrange("p (t n) -> p t n", t=2), in_=fc)

            prev_gix = None
            fA = None
            for _rep in range(repeat):
                if fA is None:
                    fA = [con_fetch(0), con_fetch(1)]

                hs = []
                gixs = []
                for p in range(NP):
                    if p + 2 < NP:
                        fA.append(con_fetch(p + 2))
                    hs.append(con_stage(p, fA[p], None))
                    if KNOBS["co_ilv"] and p >= 1:
                        gixs.append(co_stage(p - 1, hs[p - 1][0], None, None))
                fC = [gen_fetch(0), gen_fetch(1)]
                if KNOBS["co_ilv"]:
                    gixs.append(co_stage(NP - 1, hs[NP - 1][0], None, None))
                else:
                    gixs = [co_stage(p, hs[p][0], None, None)
                            for p in range(NP)]
                nextA = []
                hgs = []
                fd = 2 if KNOBS["fac_delay"] else NP
                for p in range(NP):
                    if p + 2 < NP:
                        fC.append(gen_fetch(p + 2))
                    elif _rep + 1 < repeat:
                        nextA.append(con_fetch(p + 2 - NP))
                    hgs.append(gen_stage(p, gixs[p], fC[p]))
                    if p >= fd:
                        fac_stage(p - fd, hgs[p - fd])
                for p in range(max(0, NP - fd), NP):
                    fac_stage(p, hgs[p])
                fA = nextA if nextA else None

    nc.compile()
    return nc


# ---------------------------------------------------------------------------
# host-side prep
# ---------------------------------------------------------------------------
def _enc_stream(arr_t, dt, P=None):
    """[rows, B_CORE] feature-major -> tile-pair-major
    [NP, 128orP, c*2*NT] with rows = c*P + p, cols = pair*PW + off."""
    rows, B = arr_t.shape
    if P is None:
        P = min(rows, 128)
    c = rows // P
    assert c * P == rows
    a = arr_t.reshape(c, P, NP, PW)        # [c, p, pair, t*n]
    a = a.transpose(2, 1, 0, 3)            # [pair, p, c, t*n]
    return np.ascontiguousarray(a.reshape(NP, P, c * PW).astype(dt))


def _dec_stream(a, rows):
    """Inverse of _enc_stream: [NP, P, c*PW] f32 -> [rows, B_CORE]."""
    NP_, P, flat = a.shape
    c = rows // P
    a = a.reshape(NP_, P, c, PW).transpose(2, 1, 0, 3)  # [c, p, pair, t*n]
    return a.reshape(rows, NP_ * PW)


def _prep_inputs(input, h_0, eps, gen_w_ih, gen_w_hh, con_w_ih, con_w_hh,
                 co_w, fac_w, biases):
    import ml_dtypes
    f = np.float32
    BF_NP = ml_dtypes.bfloat16
    F8_NP = ml_dtypes.float8_e4m3

    def q8(x):
        return np.asarray(x, f).astype(F8_NP).astype(f)

    input = np.asarray(input, f)
    h_0 = np.asarray(h_0, f)
    eps = np.asarray(eps, f)
    gen_w_ih = np.asarray(gen_w_ih, f)
    gen_w_hh = np.asarray(gen_w_hh, f)
    con_w_ih = np.asarray(con_w_ih, f)
    con_w_hh = np.asarray(con_w_hh, f)
    co_w = np.asarray(co_w, f)
    fac_w = np.asarray(fac_w, f)

    norm = np.maximum(np.linalg.norm(fac_w, axis=1, keepdims=True), 1e-12)
    fac_wn = fac_w / norm

    def drpack(w_t, dt):
        # w_t: [K, M] feature-major weight -> [128, K//128, M],
        # element [p, c, m] = w_t[c*128 + p, m]
        K, M = w_t.shape
        return np.ascontiguousarray(
            w_t.reshape(K // 128, 128, M).transpose(1, 0, 2).astype(dt))

    H_FAC = GEN + CON + 3 * CO + EXT
    per_core = {}
    for c in range(N_CORES):
        rows = slice(c * B_CORE, (c + 1) * B_CORE)
        xT = input[rows].T
        hT = h_0[rows].T
        conx = np.concatenate([xT[0:256], hT[H_FAC:H_FAC + 128]], axis=0)
        cx8 = q8(conx)
        hcon = hT[GEN:GEN + CON]
        hgen = hT[0:GEN]
        cxh = np.concatenate([cx8, conx - cx8, hcon], axis=0)  # 1024 rows
        per_core.setdefault("cxh8d", []).append(_enc_stream(cxh, F8_NP))
        per_core.setdefault("hc16d", []).append(_enc_stream(hcon, BF_NP))
        per_core.setdefault("hg16d", []).append(_enc_stream(hgen, BF_NP))
        per_core.setdefault("hg8d", []).append(_enc_stream(hgen, F8_NP))
        ext = xT[256:272]
        ext8 = q8(ext)
        gx = np.zeros((128, B_CORE), f)
        gx[0:16] = ext8
        gx[64:80] = ext - ext8
        per_core.setdefault("gx8d", []).append(_enc_stream(gx, F8_NP, P=64))
        per_core.setdefault("epsd", []).append(
            _enc_stream(eps[rows].T, BF_NP))

    def colblocks(w, blocks):
        # w: [outs, K]; blocks: list of (array [outs, 128]) -> [128, C, outs]
        return np.ascontiguousarray(
            np.stack([b.T for b in blocks], axis=1).astype(F8_NP))

    def pad128(w):
        # [outs, k<128] -> [outs, 128] zero-padded
        out = np.zeros((w.shape[0], 128), f)
        out[:, :w.shape[1]] = w
        return out

    Wxc = con_w_ih                                  # [768, 384]
    Wxc8 = q8(Wxc)
    Whc8 = q8(con_w_hh)                             # [768, 256]
    xb = [Wxc8[:512, ts(i, 128)] for i in range(3)]
    wczr8 = colblocks(None, xb + xb +
                      [Whc8[:512, ts(i, 128)] for i in range(2)])
    xbn = [Wxc8[512:, ts(i, 128)] for i in range(3)]
    wcn8 = colblocks(None, xbn + xbn +
                     [q8(con_w_hh[512:])[:, ts(i, 128)] for i in range(2)])

    Wg = gen_w_ih                                   # [1536, 80]
    Wg8 = q8(Wg)
    Wgr8 = q8(Wg - Wg8)
    Whg8 = q8(gen_w_hh[:2 * GEN])                   # [1024, 512]
    Wng8 = q8(gen_w_hh[2 * GEN:])                   # [512, 512]
    g8z = pad128(Wg8[:1024])
    gr8z = pad128(Wgr8[:1024])
    wgzr8 = colblocks(None,
                      [Whg8[:, ts(i, 128)] for i in range(4)] +
                      [g8z, g8z, gr8z, gr8z])
    g8n = pad128(Wg8[1024:])
    gr8n = pad128(Wgr8[1024:])
    wgn8 = colblocks(None, [g8n, g8n, gr8n, gr8n] +
                     [Wng8[:, ts(i, 128)] for i in range(4)])

    weights = {
        "wczr8d": wczr8,                                 # [128, 8, 512]
        "wcn8d": wcn8,                                   # [128, 8, 256]
        "wcod": drpack(co_w.T, BF_NP),                   # [128, 2, 128]
        "wgzr8d": wgzr8,                                 # [128, 8, 1024]
        "wgn8d": wgn8,                                   # [128, 8, 512]
        "wfacd": drpack(fac_wn.T, BF_NP),                # [128, 4, 128]
    }

    gen_b_ih, gen_b_hh, con_b_ih, con_b_hh, co_b = [
        np.asarray(b, f) for b in biases]
    has_bias = any(np.any(b) for b in (gen_b_ih, gen_b_hh, con_b_ih,
                                       con_b_hh, co_b))
    if has_bias:
        bc = con_b_ih + con_b_hh
        bg = gen_b_ih + gen_b_hh
        weights["bczrd"] = np.ascontiguousarray(bc[:512].reshape(4, 128).T)
        weights["bcnd"] = np.ascontiguousarray(bc[512:].reshape(2, 128).T)
        weights["bgzrd"] = np.ascontiguousarray(bg[:1024].reshape(8, 128).T)
        weights["bgnd"] = np.ascontiguousarray(bg[1024:].reshape(4, 128).T)
        weights["bcod"] = np.ascontiguousarray(
            np.stack([co_b[:64], 0.5 * co_b[64:]], axis=1))
    return per_core, weights, has_bias


def _assemble_core(outs_c, input_rows):
    """outs_c: dict name -> per-core output array (f32). Returns
    [B_CORE, H_DIM] f32 with host clip + ext passthrough."""
    out = np.empty((B_CORE, H_DIM), np.float32)
    out[:, 0:512] = _dec_stream(outs_c["od_gen"], 512).T
    out[:, 512:768] = _dec_stream(outs_c["od_con"], 256).T
    ms = _dec_stream(outs_c["od_msg"], 128)
    out[:, 768:832] = ms[0:64].T
    out[:, 832:896] = ms[64:128].T
    out[:, 896:960] = _dec_stream(outs_c["od_gi"], 64).T
    out[:, 976:1104] = _dec_stream(outs_c["od_fac"], 128).T
    np.clip(out[:, 0:768], -CLIP, CLIP, out=out[:, 0:768])
    out[:, 960:976] = input_rows[:, 256:272]
    return out


# ---------------------------------------------------------------------------
# host-side runner (cached per process)
# ---------------------------------------------------------------------------
_CACHE = {}


def _get_runner(has_bias):
    key = has_bias
    if key not in _CACHE:
        nc = build_program(has_bias)
        _CACHE[key] = _make_runner(nc)
    return _CACHE[key]


def _make_runner(nc):
    import jax
    from jax.sharding import Mesh, PartitionSpec, NamedSharding
    from jax.experimental.shard_map import shard_map
    from concourse.bass2jax import (_bass_exec_p, install_neuronx_cc_hook,
                                    partition_id_tensor)

    install_neuronx_cc_hook()
    partition_name = (nc.partition_id_tensor.name
                      if nc.partition_id_tensor else None)
    in_names, out_names, out_avals, zero_outs = [], [], [], []
    for alloc in nc.m.functions[0].allocations:
        if not isinstance(alloc, mybir.MemoryLocationSet):
            continue
        name = alloc.memorylocations[0].name
        if alloc.kind == "ExternalInput":
            if name != partition_name:
                in_names.append(name)
        elif alloc.kind == "ExternalOutput":
            shape = tuple(alloc.tensor_shape)
            dtype = mybir.dt.np(alloc.dtype)
            out_names.append(name)
            out_avals.append(jax.core.ShapedArray(shape, dtype))
            zero_outs.append(np.zeros(shape, dtype))
    all_in = in_names + out_names
    if partition_name is not None:
        all_in.append(partition_name)

    def _body(*args):
        operands = list(args)
        if partition_name is not None:
            operands.append(partition_id_tensor())
        return tuple(_bass_exec_p.bind(
            *operands, out_avals=tuple(out_avals), in_names=tuple(all_in),
            out_names=tuple(out_names),
            lowering_input_output_aliases=(),
            sim_require_finite=True, sim_require_nnan=True, nc=nc))

    devices = jax.devices()[:N_CORES]
    mesh = Mesh(np.asarray(devices), ("core",))
    nin = len(in_names)
    fn = jax.jit(
        shard_map(_body, mesh=mesh,
                  in_specs=(PartitionSpec("core"),) * (nin + len(out_names)),
                  out_specs=(PartitionSpec("core"),) * len(out_names),
                  check_rep=False),
        keep_unused=True)
    sharding = NamedSharding(mesh, PartitionSpec("core"))

    class R:
        pass

    r = R()
    r.jax = jax
    r.fn = fn
    r.sharding = sharding
    r.in_names = in_names
    r.out_names = out_names
    r.out_avals = out_avals
    r.zero_outs = zero_outs
    return r


def kernel(input, h_0, eps, gen_w_ih, gen_b_ih, gen_w_hh, gen_b_hh,
           con_w_ih, con_b_ih, con_w_hh, con_b_hh, co_w, co_b, fac_w):
    input = np.asarray(input, np.float32)
    per_core, weights, has_bias = _prep_inputs(
        input, h_0, eps, gen_w_ih, gen_w_hh, con_w_ih, con_w_hh, co_w, fac_w,
        (gen_b_ih, gen_b_hh, con_b_ih, con_b_hh, co_b))

    r = _get_runner(has_bias)
    jax = r.jax

    args = []
    for name in r.in_names:
        if name in per_core:
            a = np.concatenate(per_core[name], axis=0)
        else:
            a = np.concatenate([weights[name]] * N_CORES, axis=0)
        args.append(jax.device_put(a, r.sharding))
    for z in r.zero_outs:
        args.append(jax.device_put(
            np.zeros((N_CORES * z.shape[0], *z.shape[1:]), z.dtype),
            r.sharding))

    outs = jax.block_until_ready(r.fn(*args))
    out = np.empty((BATCH, H_DIM), np.float32)
    for c in range(N_CORES):
        rows = slice(c * B_CORE, (c + 1) * B_CORE)
        outs_c = {}
        for i, name in enumerate(r.out_names):
            a = np.asarray(outs[i]).astype(np.float32)
            pershard = a.shape[0] // N_CORES
            outs_c[name] = a[c * pershard:(c + 1) * pershard]
        out[rows] = _assemble_core(outs_c, input[rows])
    return out

